# revision 9
# baseline (speedup 1.0000x reference)
"""CfC RNN kernel for Trainium2 (8 NeuronCores, batch-data-parallel).

Model (per step, reference semantics, ts = 1.0):
    z_in = concat([x_t, h])                      # [B, I+H] = [B, 768]
    z1 = 1.7159*tanh(0.666*(z_in @ wb1.T + bb1)) # [B, 1024]
    z2 = 1.7159*tanh(0.666*(z1 @ wb2.T + bb2))   # [B, 1024]
    ff1 = tanh(z2 @ wff1.T + bff1)               # [B, 512]
    ff2 = tanh(z2 @ wff2.T + bff2)
    t   = sigmoid(z2 @ (wta+wtb).T + (bta+btb))  # ta*1+tb folded
    h'  = ff1 + t*(ff2-ff1)

Device layout is dim-major everywhere: [dim -> 128 partitions, batch -> free].
Algebraic folds (host-side):
  - store z1' = tanh(0.666*pre1)  (the 1.7159 is folded into wb2)
  - store z2' = tanh(0.666*pre2)  (the 1.7159 is folded into the head weights)
  - t_a*ts + t_b with ts=1 == one matmul with (wta+wtb), bias (bta+btb)
"""

import sys

sys.path.insert(0, "/opt/trn_rl_repo")

import numpy as np

import concourse.bass as bass
import concourse.tile as tile
from concourse import bacc, mybir
from concourse import bass_utils
from concourse.bass import ds, ts

B, T, I, H, BU = 64, 512, 256, 512, 1024
NCORES = 8
BC = B // NCORES  # batch rows per core
KX = I // 128     # 2  x K-chunks
KH = H // 128     # 4  h K-chunks
M1 = BU // 128    # 8  mm1 out tiles
K2 = BU // 128    # 8  mm2 K-chunks
M2 = BU // 128    # 8  mm2 out tiles
MH = H // 128     # 4  head out tiles

AF = mybir.ActivationFunctionType

# --- build configuration ---------------------------------------------------
CFG = dict(
    dtype_w="float32",  # weights dtype (stationary operand)
    dtype_a="float32",  # activations/x/h dtype (moving operand)
    unroll=1,           # steps per For_i iteration
    hints=False,        # hint_engines on the loop back-edge
)


def _dt(name):
    return {"float32": mybir.dt.float32, "bfloat16": mybir.dt.bfloat16,
            "float32r": mybir.dt.float32r}[name]


def build(T_steps=T, cfg=CFG):
    DTW = _dt(cfg["dtype_w"])
    DT = _dt(cfg["dtype_a"])
    nc = bacc.Bacc("TRN2", target_bir_lowering=False, debug=False,
                   num_devices=NCORES)

    f32 = mybir.dt.float32
    xT_d = nc.dram_tensor("xT", [128, T_steps, KX * BC], DT, kind="ExternalInput").ap()
    w1_d = nc.dram_tensor("w1", [128, KX + KH, BU], DTW, kind="ExternalInput").ap()
    w2_d = nc.dram_tensor("w2", [128, K2, BU], DTW, kind="ExternalInput").ap()
    wf1_d = nc.dram_tensor("wf1", [128, K2, H], DTW, kind="ExternalInput").ap()
    wf2_d = nc.dram_tensor("wf2", [128, K2, H], DTW, kind="ExternalInput").ap()
    wt_d = nc.dram_tensor("wt", [128, K2, H], DTW, kind="ExternalInput").ap()
    bias_d = nc.dram_tensor("biases", [128, 28], f32, kind="ExternalInput").ap()
    ys_d = nc.dram_tensor("ys", [T_steps, 128, KH * BC], f32, kind="ExternalOutput").ap()

    with tile.TileContext(nc) as tc:
        with tc.tile_pool(name="weights", bufs=1) as wp, \
             tc.tile_pool(name="state", bufs=1) as sp, \
             tc.tile_pool(name="acts", bufs=2) as ap_, \
             tc.tile_pool(name="psum", bufs=2, space="PSUM") as pp:
            xT = wp.tile([128, T_steps, KX * BC], DT)
            w1 = wp.tile([128, KX + KH, BU], DTW)
            w2 = wp.tile([128, K2, BU], DTW)
            wf1 = wp.tile([128, K2, H], DTW)
            wf2 = wp.tile([128, K2, H], DTW)
            wt = wp.tile([128, K2, H], DTW)
            bia = wp.tile([128, 28], f32)
            for sb_t, dr in ((xT, xT_d), (w1, w1_d), (w2, w2_d), (wf1, wf1_d),
                             (wf2, wf2_d), (wt, wt_d), (bia, bias_d)):
                nc.sync.dma_start(sb_t[:], dr[:])

            h = sp.tile([128, KH * BC], DT)      # recurrent state, dim-major
            h32 = sp.tile([128, KH * BC], f32)   # fp32 copy for output DMA
            nc.vector.memset(h[:], 0.0)
            nc.vector.memset(h32[:], 0.0)

            def step(t_idx):
                z1 = ap_.tile([128, M1 * BC], DT, tag="z1")
                z2 = ap_.tile([128, M2 * BC], DT, tag="z2")
                ff1 = ap_.tile([128, MH * BC], DT, tag="ff1")
                ff2 = ap_.tile([128, MH * BC], DT, tag="ff2")
                tt = ap_.tile([128, MH * BC], DT, tag="tt")
                z1ps = pp.tile([128, M1 * BC], f32, tag="z1ps")
                z2ps = pp.tile([128, M2 * BC], f32, tag="z2ps")
                hps = pp.tile([128, 3 * MH * BC], f32, tag="hps")

                # ---- mm1: z1pre = [x_t; h] @ wb1.T  (K = 2 x-chunks + 4 h-chunks)
                for m in range(M1):
                    for k in range(KX):
                        nc.tensor.matmul(
                            z1ps[:, ts(m, BC)],
                            w1[:, k, ts(m, 128)],
                            xT[:, ds(t_idx, 1), ts(k, BC)],
                            start=(k == 0), stop=False)
                    for k in range(KH):
                        nc.tensor.matmul(
                            z1ps[:, ts(m, BC)],
                            w1[:, KX + k, ts(m, 128)],
                            h[:, ts(k, BC)],
                            start=False, stop=(k == KH - 1))
                # z1 = tanh(0.666*pre + 0.666*bb1)
                for m in range(M1):
                    nc.scalar.activation(z1[:, ts(m, BC)], z1ps[:, ts(m, BC)],
                                         AF.Tanh, bias=bia[:, m:m + 1], scale=0.666)

                # ---- mm2: z2pre = z1 @ (1.7159*wb2).T
                for m in range(M2):
                    for k in range(K2):
                        nc.tensor.matmul(
                            z2ps[:, ts(m, BC)],
                            w2[:, k, ts(m, 128)],
                            z1[:, ts(k, BC)],
                            start=(k == 0), stop=(k == K2 - 1))
                for m in range(M2):
                    nc.scalar.activation(z2[:, ts(m, BC)], z2ps[:, ts(m, BC)],
                                         AF.Tanh, bias=bia[:, 8 + m:9 + m], scale=0.666)

                # ---- heads: ff1, ff2, t (weights pre-scaled by 1.7159)
                for hd, w_sb in enumerate((wf1, wf2, wt)):
                    for m in range(MH):
                        for k in range(K2):
                            nc.tensor.matmul(
                                hps[:, ts(hd * MH + m, BC)],
                                w_sb[:, k, ts(m, 128)],
                                z2[:, ts(k, BC)],
                                start=(k == 0), stop=(k == K2 - 1))
                for m in range(MH):
                    nc.scalar.activation(ff1[:, ts(m, BC)], hps[:, ts(m, BC)],
                                         AF.Tanh, bias=bia[:, 16 + m:17 + m])
                for m in range(MH):
                    nc.scalar.activation(ff2[:, ts(m, BC)], hps[:, ts(MH + m, BC)],
                                         AF.Tanh, bias=bia[:, 20 + m:21 + m])
                for m in range(MH):
                    nc.scalar.activation(tt[:, ts(m, BC)], hps[:, ts(2 * MH + m, BC)],
                                         AF.Sigmoid, bias=bia[:, 24 + m:25 + m])

                # ---- h' = ff1 + t*(ff2-ff1)
                for c in range(KH):
                    d = ap_.tile([128, BC], f32, tag="d")
                    e = ap_.tile([128, BC], f32, tag="e")
                    nc.vector.tensor_sub(d[:], ff2[:, ts(c, BC)], ff1[:, ts(c, BC)])
                    nc.vector.tensor_mul(e[:], d[:], tt[:, ts(c, BC)])
                    nc.vector.tensor_add(h32[:, ts(c, BC)], e[:], ff1[:, ts(c, BC)])
                if DT != f32:
                    nc.scalar.activation(h[:], h32[:], AF.Copy)
                else:
                    nc.vector.tensor_copy(h[:], h32[:])

                nc.sync.dma_start(ys_d[ds(t_idx, 1), :, :], h32[:])

            U = cfg["unroll"]
            hint = ()
            if cfg["hints"]:
                hint = (mybir.EngineType.PE, mybir.EngineType.Activation,
                        mybir.EngineType.DVE)
            with tc.For_i(0, T_steps, U, hint_engines=hint) as i:
                for u in range(U):
                    step(i + u if u else i)

    nc.compile()
    return nc


# --- host side -------------------------------------------------------------

def _chunk(w2d):
    """[K, M] row-chunked to [128, K//128, M]."""
    K, M = w2d.shape
    return np.ascontiguousarray(
        w2d.reshape(K // 128, 128, M).transpose(1, 0, 2))


def _prep(np_dt_w, np_dt_a, x, wb1, bb1, wb2, bb2, wff1, bff1, wff2, bff2, wta, bta, wtb, btb,
          T_steps=T):
    f32 = np.float32
    w1 = _chunk(wb1.T.astype(f32)).astype(np_dt_w)                    # [128, 6, 1024]
    w2 = _chunk((1.7159 * wb2).T.astype(f32)).astype(np_dt_w)         # [128, 8, 1024]
    wf1 = _chunk((1.7159 * wff1).T.astype(f32)).astype(np_dt_w)       # [128, 8, 512]
    wf2 = _chunk((1.7159 * wff2).T.astype(f32)).astype(np_dt_w)
    wt = _chunk((1.7159 * (wta + wtb)).T.astype(f32)).astype(np_dt_w)
    bias = np.zeros((128, 28), f32)
    bias[:, 0:8] = (0.666 * bb1).reshape(8, 128).T
    bias[:, 8:16] = (0.666 * bb2).reshape(8, 128).T
    bias[:, 16:20] = bff1.reshape(4, 128).T
    bias[:, 20:24] = bff2.reshape(4, 128).T
    bias[:, 24:28] = (bta + btb).reshape(4, 128).T

    in_maps = []
    for c in range(NCORES):
        xc = x[c * BC:(c + 1) * BC, :T_steps].astype(f32)           # [BC, T, I]
        xT = np.ascontiguousarray(
            xc.reshape(BC, T_steps, KX, 128).transpose(3, 1, 2, 0)
        ).reshape(128, T_steps, KX * BC).astype(np_dt_a)
        in_maps.append(dict(xT=xT, w1=w1, w2=w2, wf1=wf1, wf2=wf2, wt=wt,
                            biases=bias))
    return in_maps


_CACHE = {}
LAST_EXEC_NS = None
TRACE = False
TIME_RUNS = 3


def kernel(**inputs):
    global LAST_EXEC_NS
    import ml_dtypes

    def npdt(s):
        return {"float32": np.float32, "bfloat16": ml_dtypes.bfloat16}[s]

    key = tuple(sorted(CFG.items()))
    if key not in _CACHE:
        _CACHE[key] = build(T, CFG)
    nc = _CACHE[key]
    in_maps = _prep(npdt(CFG["dtype_w"]), npdt(CFG["dtype_a"]), **inputs)
    res = bass_utils.run_bass_kernel_spmd(nc, in_maps, core_ids=list(range(NCORES)),
                                          trace=TRACE)
    LAST_EXEC_NS = res.exec_time_ns
    if TIME_RUNS:
        import time
        walls = []
        for _ in range(TIME_RUNS):
            t0 = time.time()
            bass_utils.run_bass_kernel_spmd(nc, in_maps,
                                            core_ids=list(range(NCORES)))
            walls.append(time.time() - t0)
        # wall includes host<->device transfer + axon overhead; min is an
        # upper bound on device exec time
        LAST_EXEC_NS = int(min(walls) * 1e9)
        print(f"timed runs (wall s): {[round(w,3) for w in walls]}")
    out = np.empty((B, T, H), np.float32)
    for c in range(NCORES):
        ys = res.results[c]["ys"]                                   # [T, 128, KH*BC]
        out[c * BC:(c + 1) * BC] = (
            ys.reshape(T, 128, KH, BC).transpose(3, 0, 2, 1).reshape(BC, T, H))
    return out
